# revision 12
# baseline (speedup 1.0000x reference)
"""Trainium2 Bass kernel for nn_CompressDCT.

Computes, for x of shape (32, 64, 128, 128) fp32 and q_table (8, 8) fp32:
    blocks = x reshaped into 8x8 tiles; Y = D @ blk @ D^T per tile;
    out = clip(round(Y / q), -128, 127)  (same shape as x, fp32)

Strategy (pure data-parallel over 8 NeuronCores, x sharded along N*C):
  Using the Kronecker identity vec_row(D X D^T) = (D (x) D) vec_row(X), the
  whole blocked 2D DCT is ONE matmul with the constant 128x128 stationary
  kron(I_2, R^T), R = diag(1/vec(q)) (D (x) D): each moving column holds two
  flattened 8x8 blocks, the contraction (128) covers both (2x64), and the
  output column holds the two blocks' DCT coefficients in the same layout.

  Host side prepares the per-core input as fp16 in exactly the SBUF layout
  the matmul wants (so device DMA is pure linear), and un-permutes the int8
  result back to image layout + expands to fp32.  Device side is a simple
  3-stage pipeline per tile: DMA-in fp16 -> 8x matmul(512) -> PSUM drain
  with fp32->int8 round+saturate split across ScalarE and VectorE -> DMA-out
  int8.  HBM traffic per core: 8 MiB in + 4 MiB out (vs 32 MiB fp32 in/out).

Accuracy: fp16 quantization of x and of the stationary perturbs Y by
~2.4e-4 std; Y ~ N(0,1), so ~2e-4 of the rounded outputs flip by +-1,
rel err ~1.3e-2 < 2e-2 gate.
"""

import numpy as np

B = 8            # DCT block size
P = 128          # partitions
N_CORES = 8
FT = 2048        # moving columns per tile
IMG_PER_CORE = 256           # (32/8) * 64 images of 128x128
NCOLS = IMG_PER_CORE * 128   # two-block columns per core
NT = NCOLS // FT             # tiles per core


def _dct_matrix(n=B):
    k = np.arange(n)[:, None]
    m = np.arange(n)[None, :]
    D = np.cos(np.pi * (2 * m + 1) * k / (2 * n)) * np.sqrt(2.0 / n)
    D[0, :] /= np.sqrt(2.0)
    return D.astype(np.float64)


def _build_lhsT(q_table: np.ndarray) -> np.ndarray:
    """fp16 [128,128] stationary: out = lhsT.T @ rhs = kron(I2, R) @ rhs,
    R = diag(1/vec(q)) @ (D (x) D).  Works for arbitrary q."""
    D = _dct_matrix()
    q = np.asarray(q_table, np.float64).reshape(64)
    K = np.kron(D, D)              # vec_row(D X D^T) = K @ vec_row(X)
    R = K / q[:, None]
    lhsT = np.kron(np.eye(2), R.T)
    return np.ascontiguousarray(lhsT).astype(np.float16)


def _build_program():
    import concourse.bacc as bacc
    import concourse.mybir as mybir
    import concourse.tile as tile
    import contextlib

    nc = bacc.Bacc("TRN2", target_bir_lowering=False, debug=False,
                   num_devices=N_CORES)
    x_d = nc.dram_tensor("x", [P, NCOLS], mybir.dt.float16,
                         kind="ExternalInput").ap()
    w_d = nc.dram_tensor("w", [P, P], mybir.dt.float16,
                         kind="ExternalInput").ap()
    y_d = nc.dram_tensor("y", [P, NCOLS], mybir.dt.int8,
                         kind="ExternalOutput").ap()

    # chunk schedule over the flat column space; small tail chunks shorten
    # the end-of-kernel serial chain (last in -> mm -> cvt -> last out)
    widths = [FT] * (NCOLS // FT - 1) + [FT // 2, FT // 4, FT // 8, FT // 8]
    assert sum(widths) == NCOLS

    with tile.TileContext(nc) as tc:
        with contextlib.ExitStack() as ctx:
            consts = ctx.enter_context(tc.tile_pool(name="consts", bufs=1))
            xin = ctx.enter_context(tc.tile_pool(name="xin", bufs=8))
            yout = ctx.enter_context(tc.tile_pool(name="yout", bufs=6))
            psA = ctx.enter_context(tc.tile_pool(name="psA", bufs=2, space="PSUM"))
            psB = ctx.enter_context(tc.tile_pool(name="psB", bufs=2, space="PSUM"))

            w_sb = consts.tile([P, P], mybir.dt.float16, tag="w")
            nc.sync.dma_start(w_sb[:], w_d[:])
            zbias = consts.tile([P, 1], mybir.dt.float32, tag="zbias")
            nc.gpsimd.memset(zbias[:], 0.0)

            xf = x_d
            yf = y_d

            # in-DMA issue alternates between the two HWDGE rings (SP/ACT):
            # one DIRECT2D occupies its sequencer ~650ns, so a single ring
            # rate-limits the ramp.  First chunks are hoisted so data can
            # stream during the TileContext start boilerplate.
            in_eng = [nc.sync, nc.scalar]
            HOIST = 8
            xtiles = {}
            offs = np.cumsum([0] + widths[:-1])
            for t in range(min(HOIST, len(widths))):
                wdt = widths[t]
                x_t = xin.tile([P, wdt], mybir.dt.float16, tag="x")
                in_eng[t % 2].dma_start(x_t[:], xf[:, offs[t]:offs[t] + wdt])
                xtiles[t] = x_t

            cvt = 0  # alternate ScalarE/VectorE for the PSUM drain-converts
            for t, wdt in enumerate(widths):
                off = int(offs[t])
                if t in xtiles:
                    x_t = xtiles[t]
                else:
                    x_t = xin.tile([P, wdt], mybir.dt.float16, tag="x")
                    in_eng[t % 2].dma_start(x_t[:], xf[:, off:off + wdt])

                y8 = yout.tile([P, wdt], mybir.dt.int8, tag="y8")
                for base in range(0, wdt, 1024):
                    pw = min(1024, wdt - base)
                    pool = psA if (cvt % 2 == 0) else psB
                    ps = pool.tile([P, pw], mybir.dt.float32, tag="ps")
                    for j in range(0, pw, 512):
                        n = min(512, pw - j)
                        nc.tensor.matmul(ps[:, j:j + n], w_sb[:],
                                         x_t[:, base + j:base + j + n],
                                         start=True, stop=True)
                    # fp32 -> int8: round-half-even + saturate on either engine
                    if cvt % 2 == 0:
                        nc.scalar.activation(
                            y8[:, base:base + pw], ps[:],
                            mybir.ActivationFunctionType.Identity,
                            bias=zbias[:], scale=1.0)
                    else:
                        nc.vector.tensor_copy(y8[:, base:base + pw], ps[:])
                    cvt += 1

                # out-DMA via SWDGE on the otherwise-idle GpSimd engine
                nc.gpsimd.dma_start(yf[:, off:off + wdt], y8[:])

    nc.compile()
    return nc


_prog_cache = {}

# test-harness knobs (harmless in production: TRACE stays False)
TRACE = False
LAST_RESULT = None


def kernel(x: np.ndarray, q_table: np.ndarray) -> np.ndarray:
    global LAST_RESULT
    from concourse.bass_utils import run_bass_kernel_spmd

    x = np.asarray(x, np.float32)
    Nb, C, H, W = x.shape
    assert (H, W) == (P, P) and (Nb * C) % (N_CORES * FT // 128) == 0

    w16 = _build_lhsT(q_table)

    # host: fp16 + relayout so each device column is two flattened 8x8 blocks
    # [core, img, hb, m, wb2, s, l] -> [core, (s m l), (img hb wb2)]
    x16 = x.astype(np.float16)
    xs = x16.reshape(N_CORES, NCOLS // 128, 16, 8, 8, 2, 8)
    xd = np.ascontiguousarray(xs.transpose(0, 5, 3, 6, 1, 2, 4)) \
           .reshape(N_CORES, P, NCOLS)

    if "prog" not in _prog_cache:
        _prog_cache["prog"] = _build_program()
    nc = _prog_cache["prog"]

    in_maps = [{"x": xd[c], "w": w16} for c in range(N_CORES)]

    kwargs = {}
    if TRACE:
        kwargs = dict(trace=True, trace_cores=[0])
    res = run_bass_kernel_spmd(nc, in_maps, core_ids=list(range(N_CORES)),
                               **kwargs)
    LAST_RESULT = res

    y = np.stack([r["y"] for r in res.results], 0)  # [core, P, NCOLS] int8
    # invert: partition p = (s i j), column = (img hb wb2)
    yb = y.reshape(N_CORES, 2, 8, 8, NCOLS // 128, 16, 8)
    out = yb.transpose(0, 4, 5, 2, 6, 1, 3) \
            .reshape(Nb, C, H, W).astype(np.float32)
    return out


# revision 14
# speedup vs baseline: 1.1400x; 1.1400x over previous
"""Trainium2 Bass kernel for nn_CompressDCT.

Computes, for x of shape (32, 64, 128, 128) fp32 and q_table (8, 8) fp32:
    blocks = x reshaped into 8x8 tiles; Y = D @ blk @ D^T per tile;
    out = clip(round(Y / q), -128, 127)  (same shape as x, fp32)

Strategy (pure data-parallel over 8 NeuronCores, x sharded along N*C):
  Using the Kronecker identity vec_row(D X D^T) = (D (x) D) vec_row(X), the
  whole blocked 2D DCT is ONE matmul with the constant 128x128 stationary
  kron(I_2, R^T), R = diag(1/vec(q)) (D (x) D): each moving column holds two
  flattened 8x8 blocks, the contraction (128) covers both (2x64), and the
  output column holds the two blocks' DCT coefficients in the same layout.

  Host side prepares the per-core input as fp16 in exactly the SBUF layout
  the matmul wants (so device DMA is pure linear), and un-permutes the int8
  result back to image layout + expands to fp32.  Device side is a simple
  3-stage pipeline per tile: DMA-in fp16 -> 8x matmul(512) -> PSUM drain
  with fp32->int8 round+saturate split across ScalarE and VectorE -> DMA-out
  int8.  HBM traffic per core: 8 MiB in + 4 MiB out (vs 32 MiB fp32 in/out).

Accuracy: fp16 quantization of x and of the stationary perturbs Y by
~2.4e-4 std; Y ~ N(0,1), so ~2e-4 of the rounded outputs flip by +-1,
rel err ~1.3e-2 < 2e-2 gate.
"""

import numpy as np

B = 8            # DCT block size
P = 128          # partitions
N_CORES = 8
FT = 2048        # moving columns per tile
IMG_PER_CORE = 256           # (32/8) * 64 images of 128x128
NCOLS = IMG_PER_CORE * 128   # two-block columns per core
NT = NCOLS // FT             # tiles per core


def _dct_matrix(n=B):
    k = np.arange(n)[:, None]
    m = np.arange(n)[None, :]
    D = np.cos(np.pi * (2 * m + 1) * k / (2 * n)) * np.sqrt(2.0 / n)
    D[0, :] /= np.sqrt(2.0)
    return D.astype(np.float64)


def _build_lhsT(q_table: np.ndarray) -> np.ndarray:
    """fp16 [128,128] stationary: out = lhsT.T @ rhs = kron(I2, R) @ rhs,
    R = diag(1/vec(q)) @ (D (x) D).  Works for arbitrary q."""
    D = _dct_matrix()
    q = np.asarray(q_table, np.float64).reshape(64)
    K = np.kron(D, D)              # vec_row(D X D^T) = K @ vec_row(X)
    R = K / q[:, None]
    lhsT = np.kron(np.eye(2), R.T)
    return np.ascontiguousarray(lhsT).astype(np.float16)


def _build_program():
    import concourse.bacc as bacc
    import concourse.mybir as mybir
    import concourse.tile as tile
    import contextlib

    nc = bacc.Bacc("TRN2", target_bir_lowering=False, debug=False,
                   num_devices=N_CORES)
    x_d = nc.dram_tensor("x", [P, NCOLS], mybir.dt.float16,
                         kind="ExternalInput").ap()
    w_d = nc.dram_tensor("w", [P, P], mybir.dt.float16,
                         kind="ExternalInput").ap()
    y_d = nc.dram_tensor("y", [P, NCOLS], mybir.dt.int8,
                         kind="ExternalOutput").ap()

    # chunk schedule over the flat column space; small tail chunks shorten
    # the end-of-kernel serial chain (last in -> mm -> cvt -> last out)
    widths = [FT] * (NCOLS // FT - 1) + [FT // 2, FT // 4, FT // 8, FT // 8]
    assert sum(widths) == NCOLS

    with tile.TileContext(nc) as tc:
        with contextlib.ExitStack() as ctx:
            consts = ctx.enter_context(tc.tile_pool(name="consts", bufs=1))
            xin = ctx.enter_context(tc.tile_pool(name="xin", bufs=8))
            yout = ctx.enter_context(tc.tile_pool(name="yout", bufs=6))
            psA = ctx.enter_context(tc.tile_pool(name="psA", bufs=2, space="PSUM"))
            psB = ctx.enter_context(tc.tile_pool(name="psB", bufs=2, space="PSUM"))

            w_sb = consts.tile([P, P], mybir.dt.float16, tag="w")
            nc.sync.dma_start(w_sb[:], w_d[:])
            zbias = consts.tile([P, 1], mybir.dt.float32, tag="zbias")
            nc.gpsimd.memset(zbias[:], 0.0)

            xf = x_d
            yf = y_d

            # One DIRECT2D occupies its sequencer ~650ns.  Keep the SP ring
            # a pure in-DMA issuer (an out-DMA's sem wait would stall later
            # in-issues); out-DMAs ride the ACT ring behind their converts.
            in_eng = [nc.sync, nc.sync]
            HOIST = 8
            xtiles = {}
            offs = np.cumsum([0] + widths[:-1])
            for t in range(min(HOIST, len(widths))):
                wdt = widths[t]
                x_t = xin.tile([P, wdt], mybir.dt.float16, tag="x")
                in_eng[t % 2].dma_start(x_t[:], xf[:, offs[t]:offs[t] + wdt])
                xtiles[t] = x_t

            cvt = 0  # alternate ScalarE/VectorE for the PSUM drain-converts
            for t, wdt in enumerate(widths):
                off = int(offs[t])
                if t in xtiles:
                    x_t = xtiles[t]
                else:
                    x_t = xin.tile([P, wdt], mybir.dt.float16, tag="x")
                    in_eng[t % 2].dma_start(x_t[:], xf[:, off:off + wdt])

                y8 = yout.tile([P, wdt], mybir.dt.int8, tag="y8")
                for base in range(0, wdt, 1024):
                    pw = min(1024, wdt - base)
                    pool = psA if (cvt % 2 == 0) else psB
                    ps = pool.tile([P, pw], mybir.dt.float32, tag="ps")
                    for j in range(0, pw, 512):
                        n = min(512, pw - j)
                        nc.tensor.matmul(ps[:, j:j + n], w_sb[:],
                                         x_t[:, base + j:base + j + n],
                                         start=True, stop=True)
                    # fp32 -> int8: round-half-even + saturate on either engine
                    if cvt % 2 == 0:
                        nc.scalar.activation(
                            y8[:, base:base + pw], ps[:],
                            mybir.ActivationFunctionType.Identity,
                            bias=zbias[:], scale=1.0)
                    else:
                        nc.vector.tensor_copy(y8[:, base:base + pw], ps[:])
                    cvt += 1

                nc.scalar.dma_start(yf[:, off:off + wdt], y8[:])

    nc.compile()
    return nc


_prog_cache = {}

# test-harness knobs (harmless in production: TRACE stays False)
TRACE = False
LAST_RESULT = None


def kernel(x: np.ndarray, q_table: np.ndarray) -> np.ndarray:
    global LAST_RESULT
    from concourse.bass_utils import run_bass_kernel_spmd

    x = np.asarray(x, np.float32)
    Nb, C, H, W = x.shape
    assert (H, W) == (P, P) and (Nb * C) % (N_CORES * FT // 128) == 0

    w16 = _build_lhsT(q_table)

    # host: fp16 + relayout so each device column is two flattened 8x8 blocks
    # [core, img, hb, m, wb2, s, l] -> [core, (s m l), (img hb wb2)]
    x16 = x.astype(np.float16)
    xs = x16.reshape(N_CORES, NCOLS // 128, 16, 8, 8, 2, 8)
    xd = np.ascontiguousarray(xs.transpose(0, 5, 3, 6, 1, 2, 4)) \
           .reshape(N_CORES, P, NCOLS)

    if "prog" not in _prog_cache:
        _prog_cache["prog"] = _build_program()
    nc = _prog_cache["prog"]

    in_maps = [{"x": xd[c], "w": w16} for c in range(N_CORES)]

    kwargs = {}
    if TRACE:
        kwargs = dict(trace=True, trace_cores=[0])
    res = run_bass_kernel_spmd(nc, in_maps, core_ids=list(range(N_CORES)),
                               **kwargs)
    LAST_RESULT = res

    y = np.stack([r["y"] for r in res.results], 0)  # [core, P, NCOLS] int8
    # invert: partition p = (s i j), column = (img hb wb2)
    yb = y.reshape(N_CORES, 2, 8, 8, NCOLS // 128, 16, 8)
    out = yb.transpose(0, 4, 5, 2, 6, 1, 3) \
            .reshape(Nb, C, H, W).astype(np.float32)
    return out
